# revision 2
# baseline (speedup 1.0000x reference)
"""LoRA q/v + full self-attention (B=4, T=2048, H=768, R=64) on 8 TRN2 cores.

Wall-clock-optimized v2. The metric is end-to-end kernel() time and the
dominant cost is the axon host<->device tunnel (~100 MB/s up, ~60 MB/s down,
~70 ms/op latency), so v2 minimizes bytes moved and per-call dispatch work:

  - inputs shipped once per core as fp16 [T,H] (12.6 MB/batch-pair instead of
    the 50 MB fp32 dual-layout of v1); [H,T] layout is rebuilt on device with
    96 PE transposes (~20 us, free next to the 70 ms wire time).
  - LoRA weights fp16, packed into two tensors (wa=[A_q|A_v], wb=[B_q;B_v]).
  - output written fp16 (12.6 MB fetched, cast to fp32 on host). rel-err vs
    fp32 reference ~1e-3, well inside the 2e-2 gate; all matmuls accumulate
    in fp32 PSUM.
  - the XLA executable is compiled once and cached; per-call work is one
    batched device_put, one executable dispatch, one fetch. The donated
    fp16 output buffer is recycled from the previous call's result.

Sharding: core c handles batch b=c//2, query rows t0=(c%2)*1024..+1024. Host
rolls the sequence axis by -t0 so each core's query rows are rows 0:1024 --
the device program is identical across cores (SPMD). Attention is
order-invariant over the key axis, so the roll only changes fp summation
order.

Device program (per core):
  xT = transpose(xh)                          (PE identity-transpose, 96 tiles)
  u  = [A_q|A_v]^T @ xT                       (PE; u[0:64]=uq, u[64:128]=uv)
  qT = xT[:, :1024] + B_q^T @ uq              (PE, + I@xT accumulated in PSUM)
  v  = xh + (B_v^T @ uv)^T ; v[:,768] = 1.0   (PE, + I@xh accumulated in PSUM)
  scoresT[s,t] = sum_h xT[h,s] * qT[h,t]      (PE, PSUM over 6 h-chunks)
  attT = exp(scoresT*scale + bias[s])         (ACT; bias 0/-1e30 from mask;
                                               no max-subtraction: |s*scale|~5)
  outp[t,0:769] = sum_s attT[s,t] * v[s,:]    (PE; col 768 = softmax denom)
  out[t,:] = outp[t,0:768] / outp[t,768]      (DVE recip + ACT scaled copy)
"""

import numpy as np


def _ensure_path():
    try:
        import concourse  # noqa: F401
    except ImportError:
        import sys

        for p in ("/opt/trn_rl_repo", "/root/.axon_site/_ro/trn_rl_repo"):
            sys.path.insert(0, p)
            try:
                import concourse  # noqa: F401

                return
            except ImportError:
                sys.path.pop(0)
        raise


_ensure_path()

import concourse.bass as bass  # noqa: E402
from concourse import bacc  # noqa: E402
import concourse.tile as tile  # noqa: E402
from concourse import mybir  # noqa: E402
from concourse import masks  # noqa: E402
from concourse.vector_clock import ScopedClock, VectorClock  # noqa: E402


# --- workaround: this walrus build rejects >1 sync-wait on the TileContext
# kernel-tail drain ("Too many sync wait commands", CoreV3GenImpl.cpp:104).
# Emit one drain per busy proc, each carrying a single sem wait.
def _patched_drain_and_barrier(self, tick_clock, wait_clock):
    gc = tick_clock.global_clock
    n = len(gc)
    for p in range(n):
        t = gc[p]
        if t <= 0:
            continue
        vec = [0] * n
        vec[p] = t
        d = self.nc.sync.drain()
        wait_clock.add_sem_waits(d.ins, ScopedClock({None: VectorClock(vec)}))

    self.nc.all_engine_barrier()
    assert self.sems is not None
    popped = self.nc._tile_sem_poison_stack.pop()
    assert popped is self._sem_poison
    self.nc.clear_and_free_semaphores(list(self.sems.allocated().values()))
    self.nc.all_engine_barrier()


tile.TileContext._drain_and_barrier = _patched_drain_and_barrier

B, T, H, R = 4, 2048, 768, 64
HC = H // 128  # 6 h-chunks
SC = T // 128  # 16 s-chunks
TQ = T // 2  # 1024 query rows per core
SCALE = float(1.0 / np.sqrt(H))
FP32 = mybir.dt.float32
FP16 = mybir.dt.float16
BF16 = mybir.dt.bfloat16
Exp = mybir.ActivationFunctionType.Exp
Copy = mybir.ActivationFunctionType.Copy

LAST_RESULTS = None


def _emit(tc, nc, xh, wa, wb, mk, out):
    from contextlib import ExitStack

    with ExitStack() as ctx:
        p_xh = ctx.enter_context(tc.tile_pool(name="p_xh", bufs=1))
        p_xT = ctx.enter_context(tc.tile_pool(name="p_xT", bufs=1))
        p_q = ctx.enter_context(tc.tile_pool(name="p_q", bufs=1))
        p_v = ctx.enter_context(tc.tile_pool(name="p_v", bufs=1))
        p_att = ctx.enter_context(tc.tile_pool(name="p_att", bufs=1))
        p_w = ctx.enter_context(tc.tile_pool(name="p_w", bufs=1))
        p_u = ctx.enter_context(tc.tile_pool(name="p_u", bufs=1))
        p_o = ctx.enter_context(tc.tile_pool(name="p_o", bufs=3))
        p_r = ctx.enter_context(tc.tile_pool(name="p_r", bufs=4))

        # ---- input DMAs (all rows-contiguous: this walrus build rejects
        # sync-waits on strided DIRECT2D pseudo-DMAs) ----
        xh_sb = [p_xh.tile([128, H], FP16, name=f"xh{j}") for j in range(SC)]
        for j in range(SC):
            nc.gpsimd.dma_start(out=xh_sb[j][:, :], in_=xh[j * 128 : (j + 1) * 128, :])

        wa_sb = [p_w.tile([128, 2 * R], FP16, name=f"wa{i}") for i in range(HC)]
        for i in range(HC):
            nc.gpsimd.dma_start(out=wa_sb[i][:, :], in_=wa[i * 128 : (i + 1) * 128, :])
        wb_sb = p_w.tile([2 * R, H], FP16, name="wb")
        nc.gpsimd.dma_start(out=wb_sb[:, :], in_=wb[:, :])

        # bias[s] = (mask-1)*1e30, precomputed host-side, one [128,1] per s-chunk
        bias_t = [p_w.tile([128, 1], FP32, name=f"bias{j}") for j in range(SC)]
        for j in range(SC):
            nc.gpsimd.dma_start(out=bias_t[j][:, :], in_=mk[j : j + 1, :].rearrange("n p -> p n"))

        ident = p_w.tile([128, 128], FP16, name="ident")
        masks.make_identity(nc, ident[:, :])

        xT_sb = [p_xT.tile([128, T], FP16, name=f"xT{i}") for i in range(HC)]
        q_sb = [p_q.tile([128, TQ], FP16, name=f"q{i}") for i in range(HC)]
        # u[0:64] = uq (valid cols 0:TQ), u[64:128] = uv (all cols); the
        # uq half of cols TQ:T is computed-but-unread garbage.
        u_sb = p_u.tile([128, T], FP16, name="u_sb")

        with (
            tc.tile_pool(name="psT", bufs=2, space="PSUM") as psT,
            tc.tile_pool(name="psL", bufs=2, space="PSUM") as psL,
        ):
            # ---- xT[i][:, j*128:+128] = xh[j][:, i*128:+128]^T (PE) ----
            for j in range(SC):
                for i in range(HC):
                    pst = psT.tile([128, 128], FP16, name="pst", tag="pst")
                    nc.tensor.transpose(
                        pst[:, :], xh_sb[j][:, i * 128 : (i + 1) * 128], ident[:, :]
                    )
                    nc.scalar.copy(xT_sb[i][:, j * 128 : (j + 1) * 128], pst[:, :])

            # ---- u = [A_q|A_v]^T @ xT  (uq rows 0:64, uv rows 64:128) ----
            for tq in range(T // 512):
                ps = psL.tile([128, 512], FP32, name="psl", tag="psl")
                for i in range(HC):
                    nc.tensor.matmul(
                        ps[:, :],
                        lhsT=wa_sb[i][:, :],
                        rhs=xT_sb[i][:, tq * 512 : (tq + 1) * 512],
                        start=(i == 0),
                        stop=(i == HC - 1),
                    )
                nc.scalar.copy(u_sb[:, tq * 512 : (tq + 1) * 512], ps[:, :])

            # ---- qT = xT[:, :TQ] + B_q^T @ uq  (x added via I @ xT) ----
            for i in range(HC):
                for tq in range(TQ // 512):
                    ps = psL.tile([128, 512], FP32, name="pslq", tag="psl")
                    nc.tensor.matmul(
                        ps[:, :],
                        lhsT=wb_sb[0:R, i * 128 : (i + 1) * 128],
                        rhs=u_sb[0:R, tq * 512 : (tq + 1) * 512],
                        start=True,
                        stop=False,
                    )
                    nc.tensor.matmul(
                        ps[:, :],
                        lhsT=ident[:, :],
                        rhs=xT_sb[i][:, tq * 512 : (tq + 1) * 512],
                        start=False,
                        stop=True,
                    )
                    nc.scalar.copy(q_sb[i][:, tq * 512 : (tq + 1) * 512], ps[:, :])

            # ---- v = xh + (B_v^T @ uv)^T ; v[:,768] = 1.0 ----
            v_sb = []
            for j in range(SC):
                vj = p_v.tile([128, 772], BF16, name=f"v{j}")
                nc.vector.memset(vj[:, 768:769], 1.0)
                ps = psL.tile([128, 768], FP32, name="pslc", tag="psl")
                for h0 in (0, 512):
                    hw = 512 if h0 == 0 else 256
                    nc.tensor.matmul(
                        ps[:, h0 : h0 + hw],
                        lhsT=u_sb[R : 2 * R, j * 128 : (j + 1) * 128],
                        rhs=wb_sb[R : 2 * R, h0 : h0 + hw],
                        start=True,
                        stop=False,
                    )
                    nc.tensor.matmul(
                        ps[:, h0 : h0 + hw],
                        lhsT=ident[:, :],
                        rhs=xh_sb[j][:, h0 : h0 + hw],
                        start=False,
                        stop=True,
                    )
                nc.scalar.copy(vj[:, 0:768], ps[:, 0:768])
                v_sb.append(vj)

        # ---- attention: 2 superblocks of 512 query cols ----
        with (
            tc.tile_pool(name="ps_s", bufs=2, space="PSUM") as ps_s,
            tc.tile_pool(name="ps_o", bufs=3, space="PSUM") as ps_o,
        ):
            for SB in range(2):
                att = []
                for j in range(SC):
                    ps = ps_s.tile([128, 512], FP32, name="pss", tag="pss")
                    for i in range(HC):
                        nc.tensor.matmul(
                            ps[:, :],
                            lhsT=xT_sb[i][:, j * 128 : (j + 1) * 128],
                            rhs=q_sb[i][:, SB * 512 : (SB + 1) * 512],
                            start=(i == 0),
                            stop=(i == HC - 1),
                        )
                    attj = p_att.tile([128, 512], BF16, name=f"att{j}")
                    nc.scalar.activation(
                        attj[:, :], ps[:, :], Exp, bias=bias_t[j][:, :], scale=SCALE
                    )
                    att.append(attj)
                for pair in range(2):
                    pso = [
                        ps_o.tile([128, 772], FP32, name="pso", tag="pso") for _ in range(2)
                    ]
                    for j in range(SC):
                        for c in range(2):
                            lc = pair * 2 + c
                            nc.tensor.matmul(
                                pso[c][:, 0:512],
                                lhsT=att[j][:, lc * 128 : (lc + 1) * 128],
                                rhs=v_sb[j][:, 0:512],
                                start=(j == 0),
                                stop=(j == SC - 1),
                            )
                            nc.tensor.matmul(
                                pso[c][:, 512:769],
                                lhsT=att[j][:, lc * 128 : (lc + 1) * 128],
                                rhs=v_sb[j][:, 512:769],
                                start=(j == 0),
                                stop=(j == SC - 1),
                            )
                    for c in range(2):
                        lc = pair * 2 + c
                        tr = SB * 512 + lc * 128
                        rc = p_r.tile([128, 1], FP32, name="rc")
                        nc.vector.reciprocal(rc[:, :], pso[c][:, 768:769])
                        ob = p_o.tile([128, H], FP16, name="ob")
                        nc.scalar.activation(
                            ob[:, :], pso[c][:, 0:768], Copy, scale=rc[:, :]
                        )
                        nc.gpsimd.dma_start(out=out[tr : tr + 128, :], in_=ob[:, :])


_NC_CACHE = None


def _build_nc():
    global _NC_CACHE
    if _NC_CACHE is not None:
        return _NC_CACHE
    nc = bacc.Bacc("TRN2", target_bir_lowering=False, debug=False)
    xh = nc.dram_tensor("xh", [T, H], FP16, kind="ExternalInput").ap()
    wa = nc.dram_tensor("wa", [H, 2 * R], FP16, kind="ExternalInput").ap()
    wb = nc.dram_tensor("wb", [2 * R, H], FP16, kind="ExternalInput").ap()
    mk = nc.dram_tensor("mk", [SC, 128], FP32, kind="ExternalInput").ap()
    out = nc.dram_tensor("out", [TQ, H], FP16, kind="ExternalOutput").ap()

    import os

    linearize = bool(int(os.environ.get("KERNEL_LINEARIZE", "0")))
    with tile.TileContext(nc, linearize=linearize) as tc:
        _emit(tc, nc, xh, wa, wb, mk, out)
    nc.compile()
    _NC_CACHE = nc
    return nc


# ---- cached jax execution state ----
_EXEC = None  # (compiled, in_names, sharding, out_shape)
_OUT_BUF = None  # device buffer donated as the NEFF's output tensor


def _build_exec(nc):
    global _EXEC
    if _EXEC is not None:
        return _EXEC
    import jax
    from jax.sharding import Mesh, PartitionSpec, NamedSharding
    from jax.experimental.shard_map import shard_map
    from concourse import bass2jax

    bass2jax.install_neuronx_cc_hook()

    partition_name = nc.partition_id_tensor.name if nc.partition_id_tensor else None
    in_names, out_names, out_avals = [], [], []
    for alloc in nc.m.functions[0].allocations:
        if not isinstance(alloc, mybir.MemoryLocationSet):
            continue
        name = alloc.memorylocations[0].name
        if alloc.kind == "ExternalInput":
            if name != partition_name:
                in_names.append(name)
        elif alloc.kind == "ExternalOutput":
            out_names.append(name)
            out_avals.append(
                jax.core.ShapedArray(tuple(alloc.tensor_shape), mybir.dt.np(alloc.dtype))
            )
    n_params = len(in_names)
    n_outs = len(out_names)
    all_names = in_names + out_names
    if partition_name is not None:
        all_names = all_names + [partition_name]

    def _body(*args):
        operands = list(args)
        if partition_name is not None:
            operands.append(bass2jax.partition_id_tensor())
        outs = bass2jax._bass_exec_p.bind(
            *operands,
            out_avals=tuple(out_avals),
            in_names=tuple(all_names),
            out_names=tuple(out_names),
            lowering_input_output_aliases=(),
            sim_require_finite=True,
            sim_require_nnan=True,
            nc=nc,
        )
        return tuple(outs)

    devices = jax.devices()[:8]
    mesh = Mesh(np.asarray(devices), ("core",))
    sh = NamedSharding(mesh, PartitionSpec("core"))
    nio = n_params + n_outs
    sharded = jax.jit(
        shard_map(
            _body,
            mesh=mesh,
            in_specs=(PartitionSpec("core"),) * nio,
            out_specs=(PartitionSpec("core"),) * n_outs,
            check_rep=False,
        ),
        donate_argnums=tuple(range(n_params, nio)),
        keep_unused=True,
    )
    arg_avals = []
    for name, aval in zip(
        all_names,
        [
            jax.core.ShapedArray((8 * T, H), np.float16),
            jax.core.ShapedArray((8 * H, 2 * R), np.float16),
            jax.core.ShapedArray((8 * 2 * R, H), np.float16),
            jax.core.ShapedArray((8 * SC, 128), np.float32),
            jax.core.ShapedArray((8 * TQ, H), np.float16),
        ],
    ):
        arg_avals.append(jax.ShapeDtypeStruct(aval.shape, aval.dtype, sharding=sh))
    compiled = sharded.lower(*arg_avals).compile()
    _EXEC = (compiled, in_names, sh, mesh)
    return _EXEC


def kernel(hidden_states, mask, A_q, B_q, A_v, B_v):
    global LAST_RESULTS, _OUT_BUF
    import jax
    import jax.numpy as jnp

    x16 = np.asarray(hidden_states, dtype=np.float16)
    mask = np.asarray(mask, dtype=np.int32)

    # per-core rolled inputs, written straight into the global (8*rows) arrays
    X_up = np.empty((8 * T, H), dtype=np.float16)
    Mk_up = np.empty((8 * SC, 128), dtype=np.float32)
    mkb = (mask.astype(np.float32) - 1.0) * 1e30  # [B, T]
    for c in range(8):
        b, t0 = c // 2, (c % 2) * TQ
        X_up[c * T : c * T + (T - t0)] = x16[b, t0:]
        if t0:
            X_up[c * T + (T - t0) : (c + 1) * T] = x16[b, :t0]
        mr = np.concatenate([mkb[b, t0:], mkb[b, :t0]]) if t0 else mkb[b]
        Mk_up[c * SC : (c + 1) * SC] = mr.reshape(SC, 128)

    wa1 = np.concatenate(
        [np.asarray(A_q, np.float16), np.asarray(A_v, np.float16)], axis=1
    )  # [H, 2R]
    wb1 = np.concatenate(
        [np.asarray(B_q, np.float16), np.asarray(B_v, np.float16)], axis=0
    )  # [2R, H]
    Wa_up = np.tile(wa1, (8, 1))
    Wb_up = np.tile(wb1, (8, 1))

    nc = _build_nc()
    compiled, in_names, sh, mesh = _build_exec(nc)

    host_args = {"xh": X_up, "wa": Wa_up, "wb": Wb_up, "mk": Mk_up}
    dev_in = jax.device_put([host_args[n] for n in in_names], [sh] * len(in_names))

    if _OUT_BUF is None or _OUT_BUF.is_deleted():
        _OUT_BUF = jax.jit(
            lambda: jnp.zeros((8 * TQ, H), jnp.float16), out_shardings=sh
        )()
    (out_g,) = compiled(*dev_in, _OUT_BUF)
    out_h = np.asarray(out_g)  # [8*TQ, H] fp16
    _OUT_BUF = out_g  # recycle as next call's donated buffer

    LAST_RESULTS = None
    outp = np.empty((B, T, H), dtype=np.float32)
    for c in range(8):
        b, t0 = c // 2, (c % 2) * TQ
        outp[b, t0 : t0 + TQ] = out_h[c * TQ : (c + 1) * TQ]
    return outp
